# revision 12
# baseline (speedup 1.0000x reference)
"""SSIM(3x3 avg-pool) + L1 loss kernel for Trainium2, 8 NeuronCores.

loss = 0.85 * mean(clip((1 - ssim_map)/2, 0, 1)) + 0.15 * mean(|pred - target|)

Full inputs pred/target: (16, 1, 1024, 1024) f32. Data-parallel: 2 images per
core; each core returns per-partition partials acc[128, 4] (per image: ssim
column, l1 column); the host combines and applies means / alpha-beta weights.

Layout: flat row-stripe. Partition p holds image rows 8p-1 .. 8p+8 (8 owned
rows + 1 halo row on each side; out-of-range halos are zero rows, matching the
zero-padded 3x3 box). Each row is a 1026-wide zero-padded section along the
free dimension, so BOTH box-filter directions are free-dim shifted adds:
horizontal within a section, vertical across adjacent sections. No matmuls,
no per-block loop - one instruction spans all sections of a field.

Math (per image, box SUMS not means; /9 folded into constants):
  u = p + t, v = p - t;  Su,Sv,G,H = 3x3 box sums of u, v, u^2, v^2
  S1 = (Su^2 - Sv^2)/162   S2 = (Su^2 + Sv^2)/162   D1 = G - H   D2 = G + H
  n1*n2 = (S1 + C1)(D1/18 - S1 + C2);  d1*d2 = (S2 + C1)(D2/18 - S2 + C2)
  contrib = (1 - clamp(n1n2/d1d2, -1, 1)) * 0.5;  l1 from |v| over owned rows.

Instruction budget is what matters on this target (cost is per-instruction,
nearly size-independent, engines run in parallel): ~31 compute instructions
per image, split across vector/gpsimd/scalar engines.
"""

import sys

import numpy as np

sys.path.insert(0, "/opt/trn_rl_repo")

ALPHA = 0.85
BETA = 0.15
C1 = 0.01 ** 2
C2 = 0.03 ** 2

N_CORES = 8
IMG_H = 1024
IMG_W = 1024
N_IMG_PER_CORE = 2

R = 8              # owned rows per partition
NSEC = R + 2       # sections per field (with halo rows)
S = IMG_W + 2      # zero-padded section width
W = IMG_W

# --- custom fused DVE ops (registered into concourse.dve_ops at build) ---- #
_OP_SSIM_RAT = None    # out = (in0 + s0) * (in1*s1 - in0 + imm2)
_OP_SSIM_FINAL = None  # out = (s0 - clamp(in0*in1, s1, s0))*imm2; accum += out
_CUSTOM_OPS_OK = False


def _register_custom_ops():
    global _OP_SSIM_RAT, _OP_SSIM_FINAL, _CUSTOM_OPS_OK
    if _CUSTOM_OPS_OK:
        return
    from operator import add

    import concourse.dve_ops as dv
    from concourse.dve_spec import (
        C0, C1 as SC1, C2 as SC2, Spec, Src0, Src1, Zero, lower, maxx, minn,
    )
    from concourse.dve_uop import DveOpSpec

    def _rat_ref(in0, in1, c0, c1, c2):
        a = in0.astype(np.float32)
        return (a + c0) * (in1.astype(np.float32) - a + c1)

    def _final_ref(in0, in1, c0, c1, c2):
        z = in0.astype(np.float32) * in1.astype(np.float32)
        b = (c0 - np.clip(z, c1, c0)).astype(np.float32)
        return b, b.reshape(b.shape[0], -1).sum(axis=-1, keepdims=True)

    # Only C0/SC1 slots (the imm2-less STT struct allows 2-free-dim src1);
    # the /18 on D-fields is folded into the Square scale, the 0.5 on the
    # ssim contribution into the host combine.
    defs = [
        ("SSIM_RAT2_ANT", Spec(
            body=(Src0 + C0) * (Src1 - Src0 + SC1), reference=_rat_ref)),
        ("SSIM_FINAL2_ANT", Spec(
            body=C0 - maxx(minn(Src0 * Src1, C0), SC1),
            accum=add, accum_init=Zero, reference=_final_ref)),
    ]
    made = {}
    for name, spec in defs:
        if name not in dv._SUB_OPCODE_FOR_NAME:
            stub = dv.DveOp(name, spec, subdim=False, uops_sha={})
            dv.OPS.append(stub)
            dv._SUB_OPCODE_FOR_NAME[name] = (
                dv._CUSTOM_DVE_ROW_BASE + len(dv.OPS) - 1
            )
            dv.CUSTOM_DVE_SPECS[name] = spec
        opcode = dv._SUB_OPCODE_FOR_NAME[name]
        shas = {}
        for ver in ("v3", "v4"):
            res = DveOpSpec(
                name=name, opcode=opcode, uops=lower(spec, ver=ver),
                rd1_en=dv.has_src1(spec),
            )
            shas[ver] = res.sha(ver)
        op = dv.DveOp(name, spec, subdim=False, uops_sha=shas)
        idx = next(i for i, o in enumerate(dv.OPS) if o.name == name)
        dv.OPS[idx] = op
        dv.CUSTOM_DVE_SPECS[name] = spec
        made[name] = op
    _OP_SSIM_RAT = made["SSIM_RAT2_ANT"]
    _OP_SSIM_FINAL = made["SSIM_FINAL2_ANT"]
    _CUSTOM_OPS_OK = True


def build_program(n_img, H, W_, io_internal=False):
    """Per-core program for n_img (even) HxW images.

    DRAM "ptin": [4, 128, 2*5*S] f32 - per image m in {0,1}, chunk c in {0,1}:
    ptin[2*m+c, p, :] = [p-rows secs 5c..5c+4 | t-rows secs 5c..5c+4], each
    section 1026 wide ([0, row, 0]). For timing builds (io_internal) every
    pair re-reads the same 4 slices, so the fill is 4 instructions and the
    per-pair marginal cost is pure compute + loads.
    """
    import concourse.bacc as bacc
    import concourse.tile as tile
    from concourse import mybir

    assert n_img % 2 == 0
    f32 = mybir.dt.float32
    Alu = mybir.AluOpType
    Act = mybir.ActivationFunctionType

    npairs = n_img // 2
    CH = 2 * 5 * S                      # one chunk: 2 fields x 5 secs
    FW = NSEC * S                       # one field strip: 10 secs
    acc_cols = 4 * npairs

    _register_custom_ops()
    nc = bacc.Bacc("TRN2", target_bir_lowering=False, debug=False)

    io_kind = "Internal" if io_internal else "ExternalInput"
    ptin_d = nc.dram_tensor("ptin", [4, 128, CH], f32, kind=io_kind).ap()
    acc_d = nc.dram_tensor("acc_out", [128, acc_cols], f32,
                           kind="ExternalOutput").ap()

    with tile.TileContext(nc) as tc:
        with (
            tc.tile_pool(name="consts", bufs=1) as cpool,
            tc.tile_pool(name="fld", bufs=1) as fpool,
            tc.tile_pool(name="scr", bufs=1) as spool,
        ):
            acc = cpool.tile([128, acc_cols], f32, tag="acc")
            nc.vector.memset(acc[:, :], 0.0)

            # FLD arena: [u | v | u^2 | v^2], each NSEC sections of width S
            fld = fpool.tile([128, 4 * FW], f32, tag="fld")

            if io_internal:
                nc.vector.memset(fld[:, :], 0.03125)
                for g in range(4):
                    nc.sync.dma_start(out=ptin_d[g, :, :], in_=fld[:, 0:CH])

            for pair in range(npairs):
                for m in range(2):
                    cb = 4 * pair + 2 * m   # acc cols: cb=ssim, cb+1=l1
                    uo, vo, qo, ho = 0, FW, 2 * FW, 3 * FW

                    for c in range(2):
                        raw = spool.tile([128, CH], f32, tag="scr", name="raw")
                        nc.sync.dma_start(out=raw[:, :],
                                          in_=ptin_d[2 * m + c, :, :])
                        so = 5 * c * S
                        # u = p + t ; v = p - t  (vector / gpsimd in parallel)
                        nc.vector.tensor_add(
                            fld[:, uo + so:uo + so + 5 * S],
                            raw[:, 0:5 * S], raw[:, 5 * S:10 * S])
                        nc.gpsimd.tensor_sub(
                            fld[:, vo + so:vo + so + 5 * S],
                            raw[:, 0:5 * S], raw[:, 5 * S:10 * S])

                    # L1 partial: sum |v| over owned rows (secs 1..8). The
                    # |v| image lands in the q-slot, which the Square below
                    # overwrites anyway - no scratch, no serialization.
                    vown = fld[:, vo + S:vo + 9 * S].rearrange(
                        "p (s w) -> p s w", s=8)[:, :, 1:W + 1]
                    nc.scalar.activation(
                        fld[:, qo:qo + 8 * W].rearrange("p (s w) -> p s w", s=8),
                        vown, Act.Abs, accum_out=acc[:, cb + 1:cb + 2])

                    # q = [u^2 | v^2] / 18  (the /18 that n2/d2 need on D)
                    nc.scalar.activation(
                        fld[:, qo:qo + 2 * FW], fld[:, uo:uo + 2 * FW],
                        Act.Square, scale=float(1.0 / np.sqrt(18.0)))

                    # box filter per field: h3 into scratch, v3 back into fld.
                    # Engine split balances the vector/gpsimd queues.
                    for fi, (eh, ev, fo) in enumerate([
                        (nc.vector, nc.vector, uo), (nc.gpsimd, nc.gpsimd, vo),
                        (nc.vector, nc.gpsimd, qo), (nc.gpsimd, nc.gpsimd, ho),
                    ]):
                        x = fld[:, fo:fo + FW].rearrange(
                            "p (s w) -> p s w", s=NSEC)
                        g = spool.tile([128, NSEC * W], f32, tag="scr",
                                       name="g")
                        g3 = g[:, :].rearrange("p (s w) -> p s w", s=NSEC)
                        # g = x[j] + x[j+1]; g += x[j+2]  -> 3-tap horizontal
                        eh.tensor_add(g3[:, :, :], x[:, :, 0:W],
                                      x[:, :, 1:W + 1])
                        eh.tensor_add(g3[:, :, :], g3[:, :, :],
                                      x[:, :, 2:W + 2])
                        # vertical 3-tap from g into fld (secs 0..7 valid)
                        ev.tensor_add(x[:, 0:8, 0:W], g3[:, 0:8, :],
                                      g3[:, 2:10, :])
                        ev.tensor_add(x[:, 0:8, 0:W], x[:, 0:8, 0:W],
                                      g3[:, 1:9, :])

                    # post: valid region = secs 0..7, cols 0..W-1 per field
                    def fv(fo):
                        return fld[:, fo:fo + FW].rearrange(
                            "p (s w) -> p s w", s=NSEC)[:, 0:8, 0:W]

                    # P = Su^2/162 (in place), Q = Sv^2/162: one op over a
                    # 18-section span covering both fields (junk secs 8,9 ok)
                    span = fld[:, uo:uo + FW + 8 * S].rearrange(
                        "p (s w) -> p s w", s=NSEC + 8)
                    nc.vector.scalar_tensor_tensor(
                        span[:, :, 0:W], span[:, :, 0:W], 1.0 / 162.0,
                        span[:, :, 0:W], op0=Alu.mult, op1=Alu.mult)
                    # S2 = P + Q -> v slot ; S1 = 2P - S2 -> u slot
                    # D2 = G + H -> v^2 slot ; D1 = 2G - D2 -> u^2 slot
                    # (walrus caps STT/TT at 2 free dims - no b-axis merge)
                    Su, Sv, G, Hh = fv(uo), fv(vo), fv(qo), fv(ho)
                    nc.gpsimd.tensor_add(Sv, Su, Sv)
                    nc.vector.scalar_tensor_tensor(
                        Su, Su, 2.0, Sv, op0=Alu.mult, op1=Alu.subtract)
                    nc.gpsimd.tensor_add(Hh, G, Hh)
                    nc.vector.scalar_tensor_tensor(
                        G, G, 2.0, Hh, op0=Alu.mult, op1=Alu.subtract)
                    # Both rationals in one custom op over 18-sec spans:
                    # n1n2 at u secs 0..7, d1d2 at v secs 0..7 (junk between)
                    spq = fld[:, qo:qo + FW + 8 * S].rearrange(
                        "p (s w) -> p s w", s=NSEC + 8)
                    nc.vector._custom_dve(
                        _OP_SSIM_RAT, out=span[:, :, 0:W], in0=span[:, :, 0:W],
                        in1=spq[:, :, 0:W], s0=float(C1), s1=float(C2))
                    n1n2, d1d2 = fv(uo), fv(vo)
                    nc.vector.reciprocal_approx_fast(d1d2, d1d2)
                    nc.vector._custom_dve(
                        _OP_SSIM_FINAL, out=n1n2, in0=n1n2, in1=d1d2,
                        s0=1.0, s1=-1.0, accum_out=acc[:, cb:cb + 1])

            nc.sync.dma_start(out=acc_d[:, :], in_=acc[:, :])

    nc.compile()
    return nc


_CACHE = {}


def _get_program(n_img, H, W_):
    key = (n_img, H, W_)
    if key not in _CACHE:
        _CACHE[key] = build_program(n_img, H, W_)
    return _CACHE[key]


def _pack_inputs(pred, target):
    """pred/target [2, H, W] f32 -> packed [4, 128, 2*5*S]."""
    out = np.zeros((4, 128, 2 * 5 * S), dtype=np.float32)
    idx = 8 * np.arange(128)[:, None] + np.arange(NSEC)[None, :]  # padded rows
    for m in range(2):
        for fi, img in enumerate((pred[m], target[m])):
            padded = np.zeros((IMG_H + 2, S), dtype=np.float32)
            padded[1:IMG_H + 1, 1:W + 1] = img
            rows = padded[idx]              # [128, 10, S]
            for c in range(2):
                sl = rows[:, 5 * c:5 * c + 5, :].reshape(128, 5 * S)
                out[2 * m + c, :, fi * 5 * S:(fi + 1) * 5 * S] = sl
    return out


LAST_RESULTS = None


def kernel(pred, target):
    from concourse.bass_utils import run_bass_kernel_spmd

    global LAST_RESULTS

    pred = np.asarray(pred, dtype=np.float32).reshape(16, IMG_H, IMG_W)
    target = np.asarray(target, dtype=np.float32).reshape(16, IMG_H, IMG_W)

    nc = _get_program(N_IMG_PER_CORE, IMG_H, IMG_W)

    in_maps = []
    for c in range(N_CORES):
        sl = slice(c * N_IMG_PER_CORE, (c + 1) * N_IMG_PER_CORE)
        in_maps.append({"ptin": _pack_inputs(pred[sl], target[sl])})

    res = run_bass_kernel_spmd(nc, in_maps, list(range(N_CORES)))
    LAST_RESULTS = res
    ssim_sum = 0.0
    l1_sum = 0.0
    for r in res.results:
        acc = r["acc_out"]
        ssim_sum += float(acc[:, 0::2].sum(dtype=np.float64))
        l1_sum += float(acc[:, 1::2].sum(dtype=np.float64))
    n = 16.0 * IMG_H * IMG_W
    loss = ALPHA * (0.5 * ssim_sum / n) + BETA * (l1_sum / n)
    return np.float32(loss)


# revision 15
# speedup vs baseline: 4.6392x; 4.6392x over previous
"""SSIM(3x3 avg-pool) + L1 loss kernel for Trainium2, 8 NeuronCores.

loss = 0.85 * mean(clip((1 - ssim_map)/2, 0, 1)) + 0.15 * mean(|pred - target|)

Full inputs pred/target: (16, 1, 1024, 1024) f32. Data-parallel: 2 images per
core; each core returns per-partition partials acc[128, 4] (per image: ssim
column, l1 column); the host combines and applies means / alpha-beta weights.

Layout: flat row-stripe. Partition p holds image rows 8p-1 .. 8p+8 (8 owned
rows + 1 halo row on each side; out-of-range halos are zero rows, matching the
zero-padded 3x3 box). Each row is a 1026-wide zero-padded section along the
free dimension, so BOTH box-filter directions are free-dim shifted adds:
horizontal within a section, vertical across adjacent sections. No matmuls,
no per-block loop - one instruction spans all sections of a field.

Math (per image, box SUMS not means; /9 folded into constants):
  u = p + t, v = p - t;  Su,Sv,G,H = 3x3 box sums of u, v, u^2, v^2
  S1 = (Su^2 - Sv^2)/162   S2 = (Su^2 + Sv^2)/162   D1 = G - H   D2 = G + H
  n1*n2 = (S1 + C1)(D1/18 - S1 + C2);  d1*d2 = (S2 + C1)(D2/18 - S2 + C2)
  contrib = (1 - clamp(n1n2/d1d2, -1, 1)) * 0.5;  l1 from |v| over owned rows.

Instruction budget is what matters on this target (cost is per-instruction,
nearly size-independent, engines run in parallel): ~31 compute instructions
per image, split across vector/gpsimd/scalar engines.
"""

import sys

import numpy as np

sys.path.insert(0, "/opt/trn_rl_repo")

ALPHA = 0.85
BETA = 0.15
C1 = 0.01 ** 2
C2 = 0.03 ** 2

N_CORES = 8
IMG_H = 1024
IMG_W = 1024
N_IMG_PER_CORE = 2

R = 8              # owned rows per partition
NSEC = R + 2       # sections per field (with halo rows)
S = IMG_W + 2      # zero-padded section width
W = IMG_W

# --- custom fused DVE ops (registered into concourse.dve_ops at build) ---- #
_OP_SSIM_RAT = None    # out = (in0 + s0) * (in1*s1 - in0 + imm2)
_OP_SSIM_FINAL = None  # out = (s0 - clamp(in0*in1, s1, s0))*imm2; accum += out
_CUSTOM_OPS_OK = False


def _register_custom_ops():
    global _OP_SSIM_RAT, _OP_SSIM_FINAL, _CUSTOM_OPS_OK
    if _CUSTOM_OPS_OK:
        return
    from operator import add

    import concourse.dve_ops as dv
    from concourse.dve_spec import (
        C0, C1 as SC1, C2 as SC2, Spec, Src0, Src1, Zero, lower, maxx, minn,
    )
    from concourse.dve_uop import DveOpSpec

    def _rat_ref(in0, in1, c0, c1, c2):
        a = in0.astype(np.float32)
        return (a + c0) * (in1.astype(np.float32) - a + c1)

    def _final_ref(in0, in1, c0, c1, c2):
        z = in0.astype(np.float32) * in1.astype(np.float32)
        b = (c0 - np.clip(z, c1, c0)).astype(np.float32)
        return b, b.reshape(b.shape[0], -1).sum(axis=-1, keepdims=True)

    # Only C0/SC1 slots (the imm2-less STT struct allows 2-free-dim src1);
    # the /18 on D-fields is folded into the Square scale, the 0.5 on the
    # ssim contribution into the host combine.
    defs = [
        ("SSIM_RAT2_ANT", Spec(
            body=(Src0 + C0) * (Src1 - Src0 + SC1), reference=_rat_ref)),
        ("SSIM_FINAL2_ANT", Spec(
            body=C0 - maxx(minn(Src0 * Src1, C0), SC1),
            accum=add, accum_init=Zero, reference=_final_ref)),
    ]
    made = {}
    for name, spec in defs:
        if name not in dv._SUB_OPCODE_FOR_NAME:
            stub = dv.DveOp(name, spec, subdim=False, uops_sha={})
            dv.OPS.append(stub)
            dv._SUB_OPCODE_FOR_NAME[name] = (
                dv._CUSTOM_DVE_ROW_BASE + len(dv.OPS) - 1
            )
            dv.CUSTOM_DVE_SPECS[name] = spec
        opcode = dv._SUB_OPCODE_FOR_NAME[name]
        shas = {}
        for ver in ("v3", "v4"):
            res = DveOpSpec(
                name=name, opcode=opcode, uops=lower(spec, ver=ver),
                rd1_en=dv.has_src1(spec),
            )
            shas[ver] = res.sha(ver)
        op = dv.DveOp(name, spec, subdim=False, uops_sha=shas)
        idx = next(i for i, o in enumerate(dv.OPS) if o.name == name)
        dv.OPS[idx] = op
        dv.CUSTOM_DVE_SPECS[name] = spec
        made[name] = op
    _OP_SSIM_RAT = made["SSIM_RAT2_ANT"]
    _OP_SSIM_FINAL = made["SSIM_FINAL2_ANT"]
    _CUSTOM_OPS_OK = True


def build_program(n_img, H, W_, io_internal=False):
    """Per-core program for n_img (even) HxW images.

    DRAM "ptin": [4, 128, 2*5*S] f32 - per image m in {0,1}, chunk c in {0,1}:
    ptin[2*m+c, p, :] = [p-rows secs 5c..5c+4 | t-rows secs 5c..5c+4], each
    section 1026 wide ([0, row, 0]). For timing builds (io_internal) every
    pair re-reads the same 4 slices, so the fill is 4 instructions and the
    per-pair marginal cost is pure compute + loads.
    """
    import concourse.bacc as bacc
    import concourse.tile as tile
    from concourse import mybir

    assert n_img % 2 == 0
    f32 = mybir.dt.float32
    Alu = mybir.AluOpType
    Act = mybir.ActivationFunctionType

    npairs = n_img // 2
    CH = 2 * 5 * S                      # one chunk: 2 fields x 5 secs
    FW = NSEC * S                       # one field strip: 10 secs
    acc_cols = 4 * npairs

    _register_custom_ops()
    nc = bacc.Bacc("TRN2", target_bir_lowering=False, debug=False)

    io_kind = "Internal" if io_internal else "ExternalInput"
    ptin_d = nc.dram_tensor("ptin", [4, 128, CH], f32, kind=io_kind).ap()
    acc_d = nc.dram_tensor("acc_out", [128, acc_cols], f32,
                           kind="ExternalOutput").ap()

    with tile.TileContext(nc) as tc:
        with (
            tc.tile_pool(name="consts", bufs=1) as cpool,
            tc.tile_pool(name="fld", bufs=1) as fpool,
            tc.tile_pool(name="scr", bufs=1) as spool,
        ):
            acc = cpool.tile([128, acc_cols], f32, tag="acc")
            nc.vector.memset(acc[:, :], 0.0)

            # FLD arena: [u | v | u^2 | v^2], each NSEC sections of width S
            fld = fpool.tile([128, 4 * FW], f32, tag="fld")

            if io_internal:
                nc.vector.memset(fld[:, :], 0.03125)
                for g in range(4):
                    nc.sync.dma_start(out=ptin_d[g, :, :], in_=fld[:, 0:CH])

            for pair in range(npairs):
                for m in range(2):
                    cb = 4 * pair + 2 * m   # acc cols: cb=ssim, cb+1=l1
                    uo, vo, qo, ho = 0, FW, 2 * FW, 3 * FW

                    for c in range(2):
                        raw = spool.tile([128, CH], f32, tag="scr", name="raw")
                        nc.sync.dma_start(out=raw[:, :],
                                          in_=ptin_d[2 * m + c, :, :])
                        so = 5 * c * S
                        # u = p + t ; v = p - t  (vector / gpsimd in parallel)
                        nc.vector.tensor_add(
                            fld[:, uo + so:uo + so + 5 * S],
                            raw[:, 0:5 * S], raw[:, 5 * S:10 * S])
                        nc.gpsimd.tensor_sub(
                            fld[:, vo + so:vo + so + 5 * S],
                            raw[:, 0:5 * S], raw[:, 5 * S:10 * S])

                    # L1 partial: sum |v| over owned rows (secs 1..8)
                    vown = fld[:, vo + S:vo + 9 * S].rearrange(
                        "p (s w) -> p s w", s=8)[:, :, 1:W + 1]
                    labs = spool.tile([128, 8 * W], f32, tag="scr", name="labs")
                    nc.scalar.activation(
                        labs[:, :].rearrange("p (s w) -> p s w", s=8),
                        vown, Act.Abs, accum_out=acc[:, cb + 1:cb + 2])

                    # q = [u^2 | v^2] / 18  (the /18 that n2/d2 need on D)
                    nc.scalar.activation(
                        fld[:, qo:qo + 2 * FW], fld[:, uo:uo + 2 * FW],
                        Act.Square, scale=float(1.0 / np.sqrt(18.0)))

                    # box filter per field: h3 into scratch, v3 back into fld
                    for fi, (eh, ev, fo) in enumerate([
                        (nc.vector, nc.vector, uo), (nc.gpsimd, nc.gpsimd, vo),
                        (nc.vector, nc.vector, qo), (nc.gpsimd, nc.gpsimd, ho),
                    ]):
                        x = fld[:, fo:fo + FW].rearrange(
                            "p (s w) -> p s w", s=NSEC)
                        g = spool.tile([128, NSEC * W], f32, tag="scr",
                                       name="g")
                        g3 = g[:, :].rearrange("p (s w) -> p s w", s=NSEC)
                        # g = x[j] + x[j+1]; g += x[j+2]  -> 3-tap horizontal
                        eh.tensor_add(g3[:, :, :], x[:, :, 0:W],
                                      x[:, :, 1:W + 1])
                        eh.tensor_add(g3[:, :, :], g3[:, :, :],
                                      x[:, :, 2:W + 2])
                        # vertical 3-tap from g into fld (secs 0..7 valid)
                        ev.tensor_add(x[:, 0:8, 0:W], g3[:, 0:8, :],
                                      g3[:, 2:10, :])
                        ev.tensor_add(x[:, 0:8, 0:W], x[:, 0:8, 0:W],
                                      g3[:, 1:9, :])

                    # post: valid region = secs 0..7, cols 0..W-1 per field
                    def fv(fo):
                        return fld[:, fo:fo + FW].rearrange(
                            "p (s w) -> p s w", s=NSEC)[:, 0:8, 0:W]

                    # P = Su^2/162 (in place), Q = Sv^2/162: one op over a
                    # 18-section span covering both fields (junk secs 8,9 ok)
                    span = fld[:, uo:uo + FW + 8 * S].rearrange(
                        "p (s w) -> p s w", s=NSEC + 8)
                    nc.vector.scalar_tensor_tensor(
                        span[:, :, 0:W], span[:, :, 0:W], 1.0 / 162.0,
                        span[:, :, 0:W], op0=Alu.mult, op1=Alu.mult)
                    # S2 = P + Q -> v slot ; S1 = 2P - S2 -> u slot
                    Su, Sv, G, Hh = fv(uo), fv(vo), fv(qo), fv(ho)
                    nc.vector.tensor_add(Sv, Su, Sv)
                    nc.vector.scalar_tensor_tensor(
                        Su, Su, 2.0, Sv, op0=Alu.mult, op1=Alu.subtract)
                    # D2 = G + H -> v^2 slot ; D1 = 2G - D2 -> u^2 slot
                    # (gpsimd lacks the TensorScalarPtr opcode; stt on vector)
                    nc.gpsimd.tensor_add(Hh, G, Hh)
                    nc.vector.scalar_tensor_tensor(
                        G, G, 2.0, Hh, op0=Alu.mult, op1=Alu.subtract)

                    S1, S2, D1, D2 = Su, Sv, G, Hh
                    nc.vector._custom_dve(
                        _OP_SSIM_RAT, out=S1, in0=S1, in1=D1,
                        s0=float(C1), s1=float(C2))
                    nc.vector._custom_dve(
                        _OP_SSIM_RAT, out=S2, in0=S2, in1=D2,
                        s0=float(C1), s1=float(C2))
                    nc.vector.reciprocal_approx_fast(S2, S2)
                    nc.vector._custom_dve(
                        _OP_SSIM_FINAL, out=S1, in0=S1, in1=S2,
                        s0=1.0, s1=-1.0, accum_out=acc[:, cb:cb + 1])

            nc.sync.dma_start(out=acc_d[:, :], in_=acc[:, :])

    nc.compile()
    return nc


_CACHE = {}


def _get_program(n_img, H, W_):
    key = (n_img, H, W_)
    if key not in _CACHE:
        _CACHE[key] = build_program(n_img, H, W_)
    return _CACHE[key]


def _pack_inputs(pred, target):
    """pred/target [2, H, W] f32 -> packed [4, 128, 2*5*S]."""
    out = np.zeros((4, 128, 2 * 5 * S), dtype=np.float32)
    idx = 8 * np.arange(128)[:, None] + np.arange(NSEC)[None, :]  # padded rows
    for m in range(2):
        for fi, img in enumerate((pred[m], target[m])):
            padded = np.zeros((IMG_H + 2, S), dtype=np.float32)
            padded[1:IMG_H + 1, 1:W + 1] = img
            rows = padded[idx]              # [128, 10, S]
            for c in range(2):
                sl = rows[:, 5 * c:5 * c + 5, :].reshape(128, 5 * S)
                out[2 * m + c, :, fi * 5 * S:(fi + 1) * 5 * S] = sl
    return out


LAST_RESULTS = None


def kernel(pred, target):
    from concourse.bass_utils import run_bass_kernel_spmd

    global LAST_RESULTS

    pred = np.asarray(pred, dtype=np.float32).reshape(16, IMG_H, IMG_W)
    target = np.asarray(target, dtype=np.float32).reshape(16, IMG_H, IMG_W)

    nc = _get_program(N_IMG_PER_CORE, IMG_H, IMG_W)

    in_maps = []
    for c in range(N_CORES):
        sl = slice(c * N_IMG_PER_CORE, (c + 1) * N_IMG_PER_CORE)
        in_maps.append({"ptin": _pack_inputs(pred[sl], target[sl])})

    res = run_bass_kernel_spmd(nc, in_maps, list(range(N_CORES)))
    LAST_RESULTS = res
    ssim_sum = 0.0
    l1_sum = 0.0
    for r in res.results:
        acc = r["acc_out"]
        ssim_sum += float(acc[:, 0::2].sum(dtype=np.float64))
        l1_sum += float(acc[:, 1::2].sum(dtype=np.float64))
    n = 16.0 * IMG_H * IMG_W
    loss = ALPHA * (0.5 * ssim_sum / n) + BETA * (l1_sum / n)
    return np.float32(loss)


# revision 19
# speedup vs baseline: 4.6933x; 1.0117x over previous
"""SSIM(3x3 avg-pool) + L1 loss kernel for Trainium2, 8 NeuronCores.

loss = 0.85 * mean(clip((1 - ssim_map)/2, 0, 1)) + 0.15 * mean(|pred - target|)

Full inputs pred/target: (16, 1, 1024, 1024) f32. Data-parallel: 2 images per
core; each core returns per-partition partials acc[128, 4] (per image: ssim
column, l1 column); the host combines and applies means / alpha-beta weights.

Layout: flat row-stripe. Partition p holds image rows 8p-1 .. 8p+8 (8 owned
rows + 1 halo row on each side; out-of-range halos are zero rows, matching the
zero-padded 3x3 box). Each row is a 1026-wide zero-padded section along the
free dimension, so BOTH box-filter directions are free-dim shifted adds:
horizontal within a section, vertical across adjacent sections. No matmuls,
no per-block loop - one instruction spans all sections of a field.

Math (per image, box SUMS not means; /9 folded into constants):
  u = p + t, v = p - t;  Su,Sv,G,H = 3x3 box sums of u, v, u^2, v^2
  S1 = (Su^2 - Sv^2)/162   S2 = (Su^2 + Sv^2)/162   D1 = G - H   D2 = G + H
  n1*n2 = (S1 + C1)(D1/18 - S1 + C2);  d1*d2 = (S2 + C1)(D2/18 - S2 + C2)
  contrib = (1 - clamp(n1n2/d1d2, -1, 1)) * 0.5;  l1 from |v| over owned rows.

Instruction budget is what matters on this target (cost is per-instruction,
nearly size-independent, engines run in parallel): ~31 compute instructions
per image, split across vector/gpsimd/scalar engines.
"""

import sys

import numpy as np

sys.path.insert(0, "/opt/trn_rl_repo")

ALPHA = 0.85
BETA = 0.15
C1 = 0.01 ** 2
C2 = 0.03 ** 2

N_CORES = 8
IMG_H = 1024
IMG_W = 1024
N_IMG_PER_CORE = 2

R = 8              # owned rows per partition
NSEC = R + 2       # sections per field (with halo rows)
S = IMG_W + 2      # zero-padded section width
W = IMG_W

# --- custom fused DVE ops (registered into concourse.dve_ops at build) ---- #
_OP_SSIM_RAT = None    # out = (in0 + s0) * (in1*s1 - in0 + imm2)
_OP_SSIM_FINAL = None  # out = (s0 - clamp(in0*in1, s1, s0))*imm2; accum += out
_CUSTOM_OPS_OK = False


def _register_custom_ops():
    global _OP_SSIM_RAT, _OP_SSIM_FINAL, _CUSTOM_OPS_OK
    if _CUSTOM_OPS_OK:
        return
    from operator import add

    import concourse.dve_ops as dv
    from concourse.dve_spec import (
        C0, C1 as SC1, C2 as SC2, Spec, Src0, Src1, Zero, lower, maxx, minn,
    )
    from concourse.dve_uop import DveOpSpec

    def _rat_ref(in0, in1, c0, c1, c2):
        a = in0.astype(np.float32)
        return (a + c0) * (in1.astype(np.float32) - a + c1)

    def _final_ref(in0, in1, c0, c1, c2):
        z = in0.astype(np.float32) * in1.astype(np.float32)
        b = (c0 - np.clip(z, c1, c0)).astype(np.float32)
        return b, b.reshape(b.shape[0], -1).sum(axis=-1, keepdims=True)

    # Only C0/SC1 slots (the imm2-less STT struct allows 2-free-dim src1);
    # the /18 on D-fields is folded into the Square scale, the 0.5 on the
    # ssim contribution into the host combine.
    defs = [
        ("SSIM_RAT2_ANT", Spec(
            body=(Src0 + C0) * (Src1 - Src0 + SC1), reference=_rat_ref)),
        ("SSIM_FINAL2_ANT", Spec(
            body=C0 - maxx(minn(Src0 * Src1, C0), SC1),
            accum=add, accum_init=Zero, reference=_final_ref)),
    ]
    made = {}
    for name, spec in defs:
        if name not in dv._SUB_OPCODE_FOR_NAME:
            stub = dv.DveOp(name, spec, subdim=False, uops_sha={})
            dv.OPS.append(stub)
            dv._SUB_OPCODE_FOR_NAME[name] = (
                dv._CUSTOM_DVE_ROW_BASE + len(dv.OPS) - 1
            )
            dv.CUSTOM_DVE_SPECS[name] = spec
        opcode = dv._SUB_OPCODE_FOR_NAME[name]
        shas = {}
        for ver in ("v3", "v4"):
            res = DveOpSpec(
                name=name, opcode=opcode, uops=lower(spec, ver=ver),
                rd1_en=dv.has_src1(spec),
            )
            shas[ver] = res.sha(ver)
        op = dv.DveOp(name, spec, subdim=False, uops_sha=shas)
        idx = next(i for i, o in enumerate(dv.OPS) if o.name == name)
        dv.OPS[idx] = op
        dv.CUSTOM_DVE_SPECS[name] = spec
        made[name] = op
    _OP_SSIM_RAT = made["SSIM_RAT2_ANT"]
    _OP_SSIM_FINAL = made["SSIM_FINAL2_ANT"]
    _CUSTOM_OPS_OK = True


def build_program(n_img, H, W_, io_internal=False):
    """Per-core program for n_img (even) HxW images.

    DRAM "ptin": [4, 128, 2*5*S] f32 - per image m in {0,1}, chunk c in {0,1}:
    ptin[2*m+c, p, :] = [p-rows secs 5c..5c+4 | t-rows secs 5c..5c+4], each
    section 1026 wide ([0, row, 0]). For timing builds (io_internal) every
    pair re-reads the same 4 slices, so the fill is 4 instructions and the
    per-pair marginal cost is pure compute + loads.
    """
    import concourse.bacc as bacc
    import concourse.tile as tile
    from concourse import mybir

    assert n_img % 2 == 0
    f32 = mybir.dt.float32
    bf16 = mybir.dt.bfloat16
    Alu = mybir.AluOpType
    Act = mybir.ActivationFunctionType

    npairs = n_img // 2
    CH = 2 * NSEC * S                   # one image: 2 fields x 10 secs (bf16)
    FW = NSEC * S                       # one field strip: 10 secs
    acc_cols = 4 * npairs

    _register_custom_ops()
    nc = bacc.Bacc("TRN2", target_bir_lowering=False, debug=False)

    io_kind = "Internal" if io_internal else "ExternalInput"
    ptin_d = nc.dram_tensor("ptin", [2, 128, CH], bf16, kind=io_kind).ap()
    acc_d = nc.dram_tensor("acc_out", [128, acc_cols], f32,
                           kind="ExternalOutput").ap()

    with tile.TileContext(nc) as tc:
        with (
            tc.tile_pool(name="consts", bufs=1) as cpool,
            tc.tile_pool(name="fld", bufs=1) as fpool,
            tc.tile_pool(name="scr", bufs=1) as spool,
        ):
            acc = cpool.tile([128, acc_cols], f32, tag="acc")
            nc.vector.memset(acc[:, :], 0.0)

            # FLD arena: [u | v | u^2 | v^2], each NSEC sections of width S
            fld = fpool.tile([128, 4 * FW], f32, tag="fld")

            if io_internal:
                rawf = spool.tile([128, CH], bf16, tag="scr", name="rawf")
                nc.vector.memset(rawf[:, :], 0.03125)
                for g in range(2):
                    nc.sync.dma_start(out=ptin_d[g, :, :], in_=rawf[:, :])

            for pair in range(npairs):
                for m in range(2):
                    cb = 4 * pair + 2 * m   # acc cols: cb=ssim, cb+1=l1
                    uo, vo, qo, ho = 0, FW, 2 * FW, 3 * FW

                    # one bf16 load per image; u/v computed f32 from bf16
                    raw = spool.tile([128, CH], bf16, tag="scr", name="raw")
                    nc.sync.dma_start(out=raw[:, :], in_=ptin_d[m, :, :])
                    nc.vector.tensor_add(
                        fld[:, uo:uo + FW], raw[:, 0:FW], raw[:, FW:2 * FW])
                    nc.gpsimd.tensor_sub(
                        fld[:, vo:vo + FW], raw[:, 0:FW], raw[:, FW:2 * FW])

                    # L1 partial: sum |v| over owned rows (secs 1..8)
                    vown = fld[:, vo + S:vo + 9 * S].rearrange(
                        "p (s w) -> p s w", s=8)[:, :, 1:W + 1]
                    labs = spool.tile([128, 8 * W], f32, tag="scr", name="labs")
                    nc.scalar.activation(
                        labs[:, :].rearrange("p (s w) -> p s w", s=8),
                        vown, Act.Abs, accum_out=acc[:, cb + 1:cb + 2])

                    # q = [u^2 | v^2] / 18  (the /18 that n2/d2 need on D)
                    nc.scalar.activation(
                        fld[:, qo:qo + 2 * FW], fld[:, uo:uo + 2 * FW],
                        Act.Square, scale=float(1.0 / np.sqrt(18.0)))

                    # box filter per field: h3 into scratch, v3 back into fld
                    for fi, (eh, ev, fo) in enumerate([
                        (nc.vector, nc.vector, uo), (nc.gpsimd, nc.gpsimd, vo),
                        (nc.vector, nc.vector, qo), (nc.gpsimd, nc.gpsimd, ho),
                    ]):
                        x = fld[:, fo:fo + FW].rearrange(
                            "p (s w) -> p s w", s=NSEC)
                        g = spool.tile([128, NSEC * W], f32, tag="scr",
                                       name="g")
                        g3 = g[:, :].rearrange("p (s w) -> p s w", s=NSEC)
                        # g = x[j] + x[j+1]; g += x[j+2]  -> 3-tap horizontal
                        eh.tensor_add(g3[:, :, :], x[:, :, 0:W],
                                      x[:, :, 1:W + 1])
                        eh.tensor_add(g3[:, :, :], g3[:, :, :],
                                      x[:, :, 2:W + 2])
                        # vertical 3-tap from g into fld (secs 0..7 valid)
                        ev.tensor_add(x[:, 0:8, 0:W], g3[:, 0:8, :],
                                      g3[:, 2:10, :])
                        ev.tensor_add(x[:, 0:8, 0:W], x[:, 0:8, 0:W],
                                      g3[:, 1:9, :])

                    # post: valid region = secs 0..7, cols 0..W-1 per field
                    def fv(fo):
                        return fld[:, fo:fo + FW].rearrange(
                            "p (s w) -> p s w", s=NSEC)[:, 0:8, 0:W]

                    # P = Su^2/162 (in place), Q = Sv^2/162: one op over a
                    # 18-section span covering both fields (junk secs 8,9 ok)
                    span = fld[:, uo:uo + FW + 8 * S].rearrange(
                        "p (s w) -> p s w", s=NSEC + 8)
                    nc.vector.scalar_tensor_tensor(
                        span[:, :, 0:W], span[:, :, 0:W], 1.0 / 162.0,
                        span[:, :, 0:W], op0=Alu.mult, op1=Alu.mult)
                    # S2 = P + Q -> v slot ; S1 = 2P - S2 -> u slot
                    Su, Sv, G, Hh = fv(uo), fv(vo), fv(qo), fv(ho)
                    nc.vector.tensor_add(Sv, Su, Sv)
                    nc.vector.scalar_tensor_tensor(
                        Su, Su, 2.0, Sv, op0=Alu.mult, op1=Alu.subtract)
                    # D2 = G + H -> v^2 slot ; D1 = 2G - D2 -> u^2 slot
                    # (gpsimd lacks the TensorScalarPtr opcode; stt on vector)
                    nc.gpsimd.tensor_add(Hh, G, Hh)
                    nc.vector.scalar_tensor_tensor(
                        G, G, 2.0, Hh, op0=Alu.mult, op1=Alu.subtract)

                    S1, S2, D1, D2 = Su, Sv, G, Hh
                    nc.vector._custom_dve(
                        _OP_SSIM_RAT, out=S1, in0=S1, in1=D1,
                        s0=float(C1), s1=float(C2))
                    nc.vector._custom_dve(
                        _OP_SSIM_RAT, out=S2, in0=S2, in1=D2,
                        s0=float(C1), s1=float(C2))
                    nc.vector.reciprocal_approx_fast(S2, S2)
                    nc.vector._custom_dve(
                        _OP_SSIM_FINAL, out=S1, in0=S1, in1=S2,
                        s0=1.0, s1=-1.0, accum_out=acc[:, cb:cb + 1])

            nc.sync.dma_start(out=acc_d[:, :], in_=acc[:, :])

    nc.compile()
    return nc


_CACHE = {}


def _get_program(n_img, H, W_):
    key = (n_img, H, W_)
    if key not in _CACHE:
        _CACHE[key] = build_program(n_img, H, W_)
    return _CACHE[key]


def _pack_inputs(pred, target):
    """pred/target [2, H, W] f32 -> packed bf16 [2, 128, 2*NSEC*S]."""
    import ml_dtypes

    out = np.zeros((2, 128, 2 * NSEC * S), dtype=ml_dtypes.bfloat16)
    idx = 8 * np.arange(128)[:, None] + np.arange(NSEC)[None, :]  # padded rows
    for m in range(2):
        for fi, img in enumerate((pred[m], target[m])):
            padded = np.zeros((IMG_H + 2, S), dtype=np.float32)
            padded[1:IMG_H + 1, 1:W + 1] = img
            rows = padded[idx]              # [128, 10, S]
            out[m, :, fi * NSEC * S:(fi + 1) * NSEC * S] = (
                rows.reshape(128, NSEC * S).astype(ml_dtypes.bfloat16))
    return out


LAST_RESULTS = None


def kernel(pred, target):
    from concourse.bass_utils import run_bass_kernel_spmd

    global LAST_RESULTS

    pred = np.asarray(pred, dtype=np.float32).reshape(16, IMG_H, IMG_W)
    target = np.asarray(target, dtype=np.float32).reshape(16, IMG_H, IMG_W)

    nc = _get_program(N_IMG_PER_CORE, IMG_H, IMG_W)

    in_maps = []
    for c in range(N_CORES):
        sl = slice(c * N_IMG_PER_CORE, (c + 1) * N_IMG_PER_CORE)
        in_maps.append({"ptin": _pack_inputs(pred[sl], target[sl])})

    res = run_bass_kernel_spmd(nc, in_maps, list(range(N_CORES)))
    LAST_RESULTS = res
    ssim_sum = 0.0
    l1_sum = 0.0
    for r in res.results:
        acc = r["acc_out"]
        ssim_sum += float(acc[:, 0::2].sum(dtype=np.float64))
        l1_sum += float(acc[:, 1::2].sum(dtype=np.float64))
    n = 16.0 * IMG_H * IMG_W
    loss = ALPHA * (0.5 * ssim_sum / n) + BETA * (l1_sum / n)
    return np.float32(loss)
